# revision 48
# baseline (speedup 1.0000x reference)
"""Trainium2 Bass kernel for nn_ConfusionModule (sparse attention + fusion MLP).

Reference computation (B=32, NDOC=4, L=512, E=512):
  x1_rep = tile(x1, (4,1,1))           # [128, L, E], row i = x1[i % 32]
  x2_r   = x2.reshape(-1, L, E)        # [128, L, E], row i = x2[i//4, i%4]
  scores = x1_rep @ x2_r^T * (1/sqrt(E)) masked to -1e9 outside
           (q < x1_len[i%32]) & (k < x2_len[i//4, i%4])
  attn   = softmax(scores)             # [128, L, L]  (output 2)
  x1_att = attn @ x2_r
  fus    = relu(concat([x1_rep - x1_att, x1_rep * x1_att]) @ fusion_w + fusion_b)
  pooled = concat([mean_l(fus).reshape(32,-1), max_l(fus).reshape(32,-1)])
  out    = relu(pooled @ out_w + out_b)  # [32, 1024]  (output 1)

Sharding: data-parallel over the 128 attention rows, 16 per core; core c
handles rows 16c..16c+15 = batches 4c..4c+3 of the output. No collectives.
"""

import os
import ml_dtypes
import numpy as np
from contextlib import ExitStack

import concourse.bass as bass
import concourse.mybir as mybir
import concourse.tile as tile
from concourse import bacc
from concourse.bass_utils import run_bass_kernel_spmd
from concourse.masks import make_identity

FP32 = mybir.dt.float32
FP32R = mybir.dt.float32r
AF = mybir.ActivationFunctionType
ALU = mybir.AluOpType

B, NDOC, L, E = 32, 4, 512, 512
NCORES = 8
RPC = 16                      # attention rows per core
SCALE = float(np.float32(1.0 / np.sqrt(E)))
NEG_BIG = -1.0e6              # additive key mask (exp underflows to exactly 0)
QT = 4                        # q tiles of 128 per row
EC = 4                        # e chunks of 128
KC = 4                        # k chunks of 128
CC = 8                        # contraction chunks of 128 in fusion matmul (2E)
GC = 32                       # contraction chunks of 128 in out matmul (8E)

_cache = {}


def build():
    nc = bacc.Bacc("TRN2", target_bir_lowering=False)

    x1t_d = nc.declare_dram_parameter("x1t", [RPC, E, L], FP32R, isOutput=False)
    x2t_d = nc.declare_dram_parameter("x2t", [RPC, E, L], FP32R, isOutput=False)
    x2n_d = nc.declare_dram_parameter("x2n", [RPC, L, E], FP32R, isOutput=False)
    km_d = nc.declare_dram_parameter("km", [RPC, L], FP32, isOutput=False)
    qm_d = nc.declare_dram_parameter("qm", [128, RPC * QT], FP32, isOutput=False)
    fw_d = nc.declare_dram_parameter("fw", [2 * E, E], FP32R, isOutput=False)
    fb_d = nc.declare_dram_parameter("fb", [128, EC], FP32, isOutput=False)
    ow_d = nc.declare_dram_parameter("ow", [8 * E, 2 * E], FP32R, isOutput=False)
    ob_d = nc.declare_dram_parameter("ob", [4, 2 * E], FP32, isOutput=False)
    attn_d = nc.declare_dram_parameter("attn", [RPC, L, L], FP32, isOutput=True)
    outv_d = nc.declare_dram_parameter("outv", [4, 2 * E], FP32, isOutput=True)

    with tile.TileContext(nc) as tc, ExitStack() as ctx:
        singles = ctx.enter_context(tc.tile_pool(name="singles", bufs=1))
        xpool = ctx.enter_context(tc.tile_pool(name="xpool", bufs=2))
        epool = ctx.enter_context(tc.tile_pool(name="epool", bufs=4))
        afpool = ctx.enter_context(tc.tile_pool(name="afpool", bufs=8))
        ptpool = ctx.enter_context(tc.tile_pool(name="ptpool", bufs=8))
        ctpool = ctx.enter_context(tc.tile_pool(name="ctpool", bufs=12))
        scpool = ctx.enter_context(tc.tile_pool(name="scpool", bufs=3))
        smpool = ctx.enter_context(tc.tile_pool(name="smpool", bufs=8))
        kmpool = ctx.enter_context(tc.tile_pool(name="kmpool", bufs=2))
        owpool = ctx.enter_context(tc.tile_pool(name="owpool", bufs=8))
        psS = ctx.enter_context(tc.tile_pool(name="psS", bufs=3, space="PSUM"))
        psPT = ctx.enter_context(tc.tile_pool(name="psPT", bufs=2, space="PSUM"))
        psBig = ctx.enter_context(tc.tile_pool(name="psBig", bufs=3, space="PSUM"))

        # --- constants ---
        ident = singles.tile([128, 128], FP32)
        make_identity(nc, ident[:])
        qm_sb = singles.tile([128, RPC * QT], FP32)
        fb_sb = singles.tile([128, EC], FP32)
        ob_sb = singles.tile([4, 2 * E], FP32)
        fw_sb = singles.tile([128, CC, E], FP32R)
        pooledT = singles.tile([128, 128], FP32)

        pooledT_r = singles.tile([128, 128], FP32R)
        out_acc = singles.tile([4, 2 * E], FP32)

        first = True
        for t in [4 * b + j for j in range(4) for b in range(4)]:
            # --- load row inputs (pre-transposed on host) ---
            x1t = xpool.tile([128, EC, L], FP32R, tag="x1t")
            x2t = xpool.tile([128, EC, L], FP32R, tag="x2t")
            x2n = xpool.tile([128, KC, E], FP32R, tag="x2n")
            if first:
                # finer slices for the first row: fill the DMA rings evenly so
                # the first matmul starts as early as possible
                for c in range(EC):
                    for hh in range(2):
                        sl = slice(hh * 256, (hh + 1) * 256)
                        nc.sync.dma_start(
                            x2t[:, c, sl], x2t_d[t, c * 128:(c + 1) * 128, sl])
                        nc.sync.dma_start(
                            x1t[:, c, sl], x1t_d[t, c * 128:(c + 1) * 128, sl])
                for c in range(EC):
                    nc.sync.dma_start(x2n[:, c, :], x2n_d[t, c * 128:(c + 1) * 128, :])
                # constants are needed later; queue them after the first row
                nc.sync.dma_start(qm_sb[:], qm_d[:])
                nc.sync.dma_start(fb_sb[:], fb_d[:])
                nc.sync.dma_start(fw_sb[:], fw_d[:].rearrange("(c p) e -> p c e", p=128))
                nc.sync.dma_start(ob_sb[:], ob_d[:])
                first = False
            else:
                for c in range(EC):
                    nc.sync.dma_start(x2t[:, c, :], x2t_d[t, c * 128:(c + 1) * 128, :])
                    nc.sync.dma_start(x1t[:, c, :], x1t_d[t, c * 128:(c + 1) * 128, :])
                    nc.sync.dma_start(x2n[:, c, :], x2n_d[t, c * 128:(c + 1) * 128, :])
            km_bc = kmpool.tile([128, L], FP32, tag="km")
            src = km_d[t]
            nc.sync.dma_start(
                km_bc[:],
                bass.AP(tensor=src.tensor, offset=src.offset,
                        ap=[[0, 128]] + list(src.ap)),
            )

            scol = smpool.tile([128, QT], FP32, tag="scol")
            e_tiles = []
            for qt in range(QT):
                # --- scores: S[q,k] = sum_e x1t[e,q] * x2t[e,k] ---
                s_ps = psS.tile([128, L], FP32, tag="S", name=f"sps_{t}_{qt}")
                for c in range(EC):
                    nc.tensor.matmul(
                        s_ps[:], x1t[:, c, qt * 128:(qt + 1) * 128], x2t[:, c, :],
                        start=(c == 0), stop=(c == EC - 1),
                    )
                # --- e = exp(SCALE * S) ---
                e_sb = epool.tile([128, L], FP32, tag="e", name=f"e_{t}_{qt}")
                nc.scalar.activation(
                    e_sb[:], s_ps[:], AF.Exp, bias=0.0, scale=SCALE,
                )
                # --- key mask + row sum: em = e * km, s = sum(em) ---
                em_sb = epool.tile([128, L], FP32, tag="em", name=f"em_{t}_{qt}")
                nc.vector.scalar_tensor_tensor(
                    out=em_sb[:], in0=e_sb[:], scalar=1.0, in1=km_bc[:],
                    op0=ALU.bypass, op1=ALU.mult,
                    accum_out=scol[:, qt:qt + 1],
                )
                e_tiles.append(em_sb)

            # --- per-row softmax fixups ([128,4] vectors) ---
            qmc = qm_sb[:, t * QT:(t + 1) * QT]
            nvec = smpool.tile([128, QT], FP32, tag="nvec")
            nc.vector.tensor_scalar(
                out=nvec[:], in0=scol[:], scalar1=0.0, scalar2=None, op0=ALU.is_gt)
            dvec = smpool.tile([128, QT], FP32, tag="dvec")
            nc.vector.tensor_scalar(
                out=dvec[:], in0=scol[:], scalar1=1e-30, scalar2=None, op0=ALU.max)
            rvec = smpool.tile([128, QT], FP32, tag="rvec")
            nc.vector.reciprocal(rvec[:], dvec[:])
            gvec = smpool.tile([128, QT], FP32, tag="gvec")
            nc.vector.tensor_tensor(
                out=gvec[:], in0=nvec[:], in1=qmc, op=ALU.mult)
            wvec = smpool.tile([128, QT], FP32, tag="wvec")
            nc.vector.tensor_tensor(
                out=wvec[:], in0=gvec[:], in1=rvec[:], op=ALU.mult)
            uvec = smpool.tile([128, QT], FP32, tag="uvec")
            nc.vector.tensor_scalar(
                out=uvec[:], in0=gvec[:], scalar1=-1.0 / L, scalar2=1.0 / L,
                op0=ALU.mult, op1=ALU.add)

            # --- attn = e * w[q] + u[q]; DMA out; transpose for P^T ---
            pt_sbs = []
            pt_pss = []
            for kc in range(KC):
                pt_pss.append(psPT.tile([128, L], FP32, tag="PT", name=f"ptps_{t}_{kc}"))
            for qt in range(QT):
                af = afpool.tile([128, L], FP32, tag="af")
                nc.scalar.activation(
                    af[:], e_tiles[qt][:], AF.Identity,
                    bias=uvec[:, qt:qt + 1], scale=wvec[:, qt:qt + 1],
                )
                nc.sync.dma_start(attn_d[t, qt * 128:(qt + 1) * 128, :], af[:])
                for kc in range(KC):
                    nc.tensor.matmul(
                        pt_pss[kc][:, qt * 128:(qt + 1) * 128],
                        af[:, kc * 128:(kc + 1) * 128], ident[:],
                        is_transpose=True, skip_group_check=True,
                        start=True, stop=True,
                    )
            for kc in range(KC):
                pt_sb = ptpool.tile([128, L], FP32R, tag="ptsb", name=f"ptsb_{t}_{kc}")
                nc.scalar.copy(pt_sb[:], pt_pss[kc][:])
                pt_sbs.append(pt_sb)

            # --- x1_att^T[e,q] = sum_k x2n[k,e] * attn[q,k] ---
            for m in range(EC):
                att_ps = psBig.tile([128, L], FP32, tag="big")
                for kc in range(KC):
                    nc.tensor.matmul(
                        att_ps[:], x2n[:, kc, m * 128:(m + 1) * 128], pt_sbs[kc][:],
                        start=(kc == 0), stop=(kc == 3),
                    )
                # --- catT chunks: (x1 - att)^T and (x1 * att)^T ---
                ct_sub = ctpool.tile([128, L], FP32R, tag="ct")
                nc.vector.tensor_tensor(
                    out=ct_sub[:], in0=x1t[:, m, :].bitcast(FP32), in1=att_ps[:],
                    op=ALU.subtract)
                ct_mul = ctpool.tile([128, L], FP32R, tag="ct")
                nc.vector.tensor_tensor(
                    out=ct_mul[:], in0=x1t[:, m, :].bitcast(FP32), in1=att_ps[:],
                    op=ALU.mult)
                if m == 0:
                    ct_tiles = []
                ct_tiles.append((ct_sub, ct_mul))

            # --- fusT[e,l] = relu(fusion_w^T @ catT + fb); pool mean/max ---
            j, b_loc = t % 4, t // 4
            for m in range(EC):
                fus_ps = psBig.tile([128, L], FP32, tag="big")
                for c in range(CC):
                    src = ct_tiles[c % EC][c // EC]
                    nc.tensor.matmul(
                        fus_ps[:], fw_sb[:, c, m * 128:(m + 1) * 128], src[:],
                        start=(c == 0), stop=(c == CC - 1),
                    )
                scr = scpool.tile([128, L], FP32, tag="scr")
                mean_col = (j * 4 + m) * 4 + b_loc
                max_col = (16 + j * 4 + m) * 4 + b_loc
                nc.scalar.activation(
                    scr[:], fus_ps[:], AF.Relu, bias=fb_sb[:, m:m + 1], scale=1.0,
                    accum_out=pooledT[:, mean_col:mean_col + 1],
                )
                nc.vector.reduce_max(
                    pooledT[:, max_col:max_col + 1], scr[:],
                    axis=mybir.AxisListType.X)

            # --- after the last row of group j: its slice of the out matmul ---
            if b_loc == 3:
                nc.vector.tensor_copy(
                    pooledT_r[:, 16 * j:16 * j + 16],
                    pooledT[:, 16 * j:16 * j + 16])
                nc.vector.tensor_copy(
                    pooledT_r[:, 64 + 16 * j:64 + 16 * j + 16],
                    pooledT[:, 64 + 16 * j:64 + 16 * j + 16])
                out_pss = [psBig.tile([4, 512], FP32, tag="big",
                                      name=f"outps_{j}_{h}") for h in range(2)]
                for gi, gc in enumerate(
                        [j * 4 + ec for ec in range(EC)]
                        + [16 + j * 4 + ec for ec in range(EC)]):
                    ow_sb = owpool.tile([128, 2 * E], FP32R, tag="ow",
                                        name=f"ow_{j}_{gi}")
                    nc.sync.dma_start(ow_sb[:], ow_d[gc * 128:(gc + 1) * 128, :])
                    for h in range(2):
                        nc.tensor.matmul(
                            out_pss[h][:], pooledT_r[:, gc * 4:(gc + 1) * 4],
                            ow_sb[:, h * 512:(h + 1) * 512],
                            start=(gi == 0), stop=(gi == 7),
                        )
                for h in range(2):
                    sl = slice(h * 512, (h + 1) * 512)
                    if j == 0:
                        nc.vector.tensor_copy(out_acc[:, sl], out_pss[h][:])
                    else:
                        nc.vector.tensor_tensor(
                            out=out_acc[:, sl], in0=out_pss[h][:],
                            in1=out_acc[:, sl], op=ALU.add)

        # --- final: out = relu(acc + ob) ---
        for h in range(2):
            o_sb = singles.tile([4, 512], FP32, tag=f"osb{h}")
            nc.vector.tensor_tensor(
                out=o_sb[:], in0=out_acc[:, h * 512:(h + 1) * 512],
                in1=ob_sb[:, h * 512:(h + 1) * 512], op=ALU.add)
            nc.vector.tensor_scalar(
                out=o_sb[:], in0=o_sb[:], scalar1=0.0, scalar2=None, op0=ALU.max)
            nc.sync.dma_start(outv_d[:, h * 512:(h + 1) * 512], o_sb[:])

    nc.compile()
    return nc


def _prep_core_inputs(c, x1, x2, x1_len, x2_len, fusion_w, fusion_b, out_w, out_b):
    r0 = 16 * c
    x1_rows = x1[(r0 % B):(r0 % B) + RPC]                      # [16, L, E]
    x2_rows = x2[4 * c:4 * c + 4].reshape(RPC, L, E)           # [16, L, E]
    x1_len_r = np.tile(x1_len, NDOC)[r0:r0 + RPC]              # [16]
    x2_len_r = x2_len.reshape(-1)[r0:r0 + RPC]                 # [16]

    pos = np.arange(L)
    km = (pos[None, :] < x2_len_r[:, None]).astype(np.float32)   # [16, 512]
    qm_rows = (pos[None, :] < x1_len_r[:, None]).astype(np.float32)  # [16, 512]
    # qm[128, 64]: col t*4+qt, partition p -> qm_rows[t, qt*128+p]
    qm = np.ascontiguousarray(
        qm_rows.reshape(RPC, QT, 128).transpose(2, 0, 1).reshape(128, RPC * QT))

    fb = np.ascontiguousarray(fusion_b.reshape(EC, 128).T)     # [128, 4]
    ow = out_w.copy()
    ow[:4 * E] *= np.float32(1.0 / L)                          # fold mean 1/L
    ob = np.broadcast_to(out_b, (4, 2 * E)).copy()

    return {
        "x1t": np.ascontiguousarray(x1_rows.transpose(0, 2, 1)),
        "x2t": np.ascontiguousarray(x2_rows.transpose(0, 2, 1)),
        "x2n": np.ascontiguousarray(x2_rows),
        "km": np.ascontiguousarray(km),
        "qm": qm,
        "fw": np.ascontiguousarray(fusion_w),
        "fb": fb,
        "ow": np.ascontiguousarray(ow),
        "ob": np.ascontiguousarray(ob),
    }


def kernel(x1, x2, x1_len, x2_len, fusion_w, fusion_b, out_w, out_b):
    x1 = np.asarray(x1, dtype=np.float32)
    x2 = np.asarray(x2, dtype=np.float32)
    x1_len = np.asarray(x1_len)
    x2_len = np.asarray(x2_len)
    fusion_w = np.asarray(fusion_w, dtype=np.float32)
    fusion_b = np.asarray(fusion_b, dtype=np.float32)
    out_w = np.asarray(out_w, dtype=np.float32)
    out_b = np.asarray(out_b, dtype=np.float32)

    if "nc" not in _cache:
        _cache["nc"] = build()
    nc = _cache["nc"]

    in_maps = [
        _prep_core_inputs(c, x1, x2, x1_len, x2_len,
                          fusion_w, fusion_b, out_w, out_b)
        for c in range(NCORES)
    ]
    core_ids = list(range(NCORES))
    trace = bool(int(os.environ.get("KERNEL_TRACE", "0")))
    res = run_bass_kernel_spmd(nc, in_maps, core_ids, trace=trace)
    _cache["last_result"] = res

    out = np.concatenate([res.results[c]["outv"] for c in range(NCORES)], axis=0)
    attn = np.concatenate([res.results[c]["attn"] for c in range(NCORES)], axis=0)
    return out, attn


# revision 49
# speedup vs baseline: 1.1133x; 1.1133x over previous
"""Trainium2 Bass kernel for nn_ConfusionModule (sparse attention + fusion MLP).

Reference computation (B=32, NDOC=4, L=512, E=512):
  x1_rep = tile(x1, (4,1,1))           # [128, L, E], row i = x1[i % 32]
  x2_r   = x2.reshape(-1, L, E)        # [128, L, E], row i = x2[i//4, i%4]
  scores = x1_rep @ x2_r^T * (1/sqrt(E)) masked to -1e9 outside
           (q < x1_len[i%32]) & (k < x2_len[i//4, i%4])
  attn   = softmax(scores)             # [128, L, L]  (output 2)
  x1_att = attn @ x2_r
  fus    = relu(concat([x1_rep - x1_att, x1_rep * x1_att]) @ fusion_w + fusion_b)
  pooled = concat([mean_l(fus).reshape(32,-1), max_l(fus).reshape(32,-1)])
  out    = relu(pooled @ out_w + out_b)  # [32, 1024]  (output 1)

Sharding: data-parallel over the 128 attention rows, 16 per core; core c
handles rows 16c..16c+15 = batches 4c..4c+3 of the output. No collectives.
"""

import os
import ml_dtypes
import numpy as np
from contextlib import ExitStack

import concourse.bass as bass
import concourse.mybir as mybir
import concourse.tile as tile
from concourse import bacc
from concourse.bass_utils import run_bass_kernel_spmd
from concourse.masks import make_identity

FP32 = mybir.dt.float32
FP32R = mybir.dt.float32r
AF = mybir.ActivationFunctionType
ALU = mybir.AluOpType

B, NDOC, L, E = 32, 4, 512, 512
NCORES = 8
RPC = 16                      # attention rows per core
SCALE = float(np.float32(1.0 / np.sqrt(E)))
NEG_BIG = -1.0e6              # additive key mask (exp underflows to exactly 0)
QT = 4                        # q tiles of 128 per row
EC = 4                        # e chunks of 128
KC = 4                        # k chunks of 128
CC = 8                        # contraction chunks of 128 in fusion matmul (2E)
GC = 32                       # contraction chunks of 128 in out matmul (8E)

_cache = {}


def build():
    nc = bacc.Bacc("TRN2", target_bir_lowering=False)

    x1t_d = nc.declare_dram_parameter("x1t", [RPC, E, L], FP32R, isOutput=False)
    x2t_d = nc.declare_dram_parameter("x2t", [RPC, E, L], FP32R, isOutput=False)
    x2n_d = nc.declare_dram_parameter("x2n", [RPC, L, E], FP32R, isOutput=False)
    km_d = nc.declare_dram_parameter("km", [RPC, L], FP32, isOutput=False)
    qm_d = nc.declare_dram_parameter("qm", [128, RPC * QT], FP32, isOutput=False)
    fw_d = nc.declare_dram_parameter("fw", [2 * E, E], FP32R, isOutput=False)
    fb_d = nc.declare_dram_parameter("fb", [128, EC], FP32, isOutput=False)
    ow_d = nc.declare_dram_parameter("ow", [8 * E, 2 * E], FP32R, isOutput=False)
    ob_d = nc.declare_dram_parameter("ob", [4, 2 * E], FP32, isOutput=False)
    attn_d = nc.declare_dram_parameter("attn", [RPC, L, L], FP32, isOutput=True)
    outv_d = nc.declare_dram_parameter("outv", [4, 2 * E], FP32, isOutput=True)

    with tile.TileContext(nc) as tc, ExitStack() as ctx:
        singles = ctx.enter_context(tc.tile_pool(name="singles", bufs=1))
        xpool = ctx.enter_context(tc.tile_pool(name="xpool", bufs=2))
        epool = ctx.enter_context(tc.tile_pool(name="epool", bufs=4))
        afpool = ctx.enter_context(tc.tile_pool(name="afpool", bufs=8))
        ptpool = ctx.enter_context(tc.tile_pool(name="ptpool", bufs=8))
        ctpool = ctx.enter_context(tc.tile_pool(name="ctpool", bufs=12))
        scpool = ctx.enter_context(tc.tile_pool(name="scpool", bufs=3))
        smpool = ctx.enter_context(tc.tile_pool(name="smpool", bufs=8))
        kmpool = ctx.enter_context(tc.tile_pool(name="kmpool", bufs=2))
        owpool = ctx.enter_context(tc.tile_pool(name="owpool", bufs=8))
        psS = ctx.enter_context(tc.tile_pool(name="psS", bufs=2, space="PSUM"))
        psPT = ctx.enter_context(tc.tile_pool(name="psPT", bufs=2, space="PSUM"))
        psBig = ctx.enter_context(tc.tile_pool(name="psBig", bufs=2, space="PSUM"))
        psOut = ctx.enter_context(tc.tile_pool(name="psOut", bufs=2, space="PSUM"))

        # --- constants ---
        ident = singles.tile([128, 128], FP32)
        make_identity(nc, ident[:])
        qm_sb = singles.tile([128, RPC * QT], FP32)
        fb_sb = singles.tile([128, EC], FP32)
        ob_sb = singles.tile([4, 2 * E], FP32)
        fw_sb = singles.tile([128, CC, E], FP32R)
        pooledT = singles.tile([128, 128], FP32)

        pooledT_r = singles.tile([128, 128], FP32R)
        out_pss = [psOut.tile([4, 512], FP32, tag="outps", name=f"outps_{h}")
                   for h in range(2)]

        first = True
        for t in [4 * b + j for j in range(4) for b in range(4)]:
            # --- load row inputs (pre-transposed on host) ---
            x1t = xpool.tile([128, EC, L], FP32R, tag="x1t")
            x2t = xpool.tile([128, EC, L], FP32R, tag="x2t")
            x2n = xpool.tile([128, KC, E], FP32R, tag="x2n")
            if first:
                # finer slices for the first row: fill the DMA rings evenly so
                # the first matmul starts as early as possible
                for c in range(EC):
                    for hh in range(2):
                        sl = slice(hh * 256, (hh + 1) * 256)
                        nc.sync.dma_start(
                            x2t[:, c, sl], x2t_d[t, c * 128:(c + 1) * 128, sl])
                        nc.sync.dma_start(
                            x1t[:, c, sl], x1t_d[t, c * 128:(c + 1) * 128, sl])
                for c in range(EC):
                    nc.sync.dma_start(x2n[:, c, :], x2n_d[t, c * 128:(c + 1) * 128, :])
                # constants are needed later; queue them after the first row
                nc.sync.dma_start(qm_sb[:], qm_d[:])
                nc.sync.dma_start(fb_sb[:], fb_d[:])
                nc.sync.dma_start(fw_sb[:], fw_d[:].rearrange("(c p) e -> p c e", p=128))
                nc.sync.dma_start(ob_sb[:], ob_d[:])
                first = False
            else:
                for c in range(EC):
                    nc.sync.dma_start(x2t[:, c, :], x2t_d[t, c * 128:(c + 1) * 128, :])
                    nc.sync.dma_start(x1t[:, c, :], x1t_d[t, c * 128:(c + 1) * 128, :])
                    nc.sync.dma_start(x2n[:, c, :], x2n_d[t, c * 128:(c + 1) * 128, :])
            km_bc = kmpool.tile([128, L], FP32, tag="km")
            src = km_d[t]
            nc.sync.dma_start(
                km_bc[:],
                bass.AP(tensor=src.tensor, offset=src.offset,
                        ap=[[0, 128]] + list(src.ap)),
            )

            scol = smpool.tile([128, QT], FP32, tag="scol")
            e_tiles = []
            for qt in range(QT):
                # --- scores: S[q,k] = sum_e x1t[e,q] * x2t[e,k] ---
                s_ps = psS.tile([128, L], FP32, tag="S", name=f"sps_{t}_{qt}")
                for c in range(EC):
                    nc.tensor.matmul(
                        s_ps[:], x1t[:, c, qt * 128:(qt + 1) * 128], x2t[:, c, :],
                        start=(c == 0), stop=(c == EC - 1),
                    )
                # --- e = exp(SCALE * S) ---
                e_sb = epool.tile([128, L], FP32, tag="e", name=f"e_{t}_{qt}")
                nc.scalar.activation(
                    e_sb[:], s_ps[:], AF.Exp, bias=0.0, scale=SCALE,
                )
                # --- key mask + row sum: em = e * km, s = sum(em) ---
                em_sb = epool.tile([128, L], FP32, tag="em", name=f"em_{t}_{qt}")
                nc.vector.scalar_tensor_tensor(
                    out=em_sb[:], in0=e_sb[:], scalar=1.0, in1=km_bc[:],
                    op0=ALU.bypass, op1=ALU.mult,
                    accum_out=scol[:, qt:qt + 1],
                )
                e_tiles.append(em_sb)

            # --- per-row softmax fixups ([128,4] vectors) ---
            qmc = qm_sb[:, t * QT:(t + 1) * QT]
            nvec = smpool.tile([128, QT], FP32, tag="nvec")
            nc.vector.tensor_scalar(
                out=nvec[:], in0=scol[:], scalar1=0.0, scalar2=None, op0=ALU.is_gt)
            dvec = smpool.tile([128, QT], FP32, tag="dvec")
            nc.vector.tensor_scalar(
                out=dvec[:], in0=scol[:], scalar1=1e-30, scalar2=None, op0=ALU.max)
            rvec = smpool.tile([128, QT], FP32, tag="rvec")
            nc.vector.reciprocal(rvec[:], dvec[:])
            gvec = smpool.tile([128, QT], FP32, tag="gvec")
            nc.vector.tensor_tensor(
                out=gvec[:], in0=nvec[:], in1=qmc, op=ALU.mult)
            wvec = smpool.tile([128, QT], FP32, tag="wvec")
            nc.vector.tensor_tensor(
                out=wvec[:], in0=gvec[:], in1=rvec[:], op=ALU.mult)
            uvec = smpool.tile([128, QT], FP32, tag="uvec")
            nc.vector.tensor_scalar(
                out=uvec[:], in0=gvec[:], scalar1=-1.0 / L, scalar2=1.0 / L,
                op0=ALU.mult, op1=ALU.add)

            # --- attn = e * w[q] + u[q]; DMA out; transpose for P^T ---
            pt_sbs = []
            pt_pss = []
            for kc in range(KC):
                pt_pss.append(psPT.tile([128, L], FP32, tag="PT", name=f"ptps_{t}_{kc}"))
            for qt in range(QT):
                af = afpool.tile([128, L], FP32, tag="af")
                nc.scalar.activation(
                    af[:], e_tiles[qt][:], AF.Identity,
                    bias=uvec[:, qt:qt + 1], scale=wvec[:, qt:qt + 1],
                )
                nc.sync.dma_start(attn_d[t, qt * 128:(qt + 1) * 128, :], af[:])
                for kc in range(KC):
                    nc.tensor.matmul(
                        pt_pss[kc][:, qt * 128:(qt + 1) * 128],
                        af[:, kc * 128:(kc + 1) * 128], ident[:],
                        is_transpose=True, skip_group_check=True,
                        start=True, stop=True,
                    )
            for kc in range(KC):
                pt_sb = ptpool.tile([128, L], FP32R, tag="ptsb", name=f"ptsb_{t}_{kc}")
                nc.scalar.copy(pt_sb[:], pt_pss[kc][:])
                pt_sbs.append(pt_sb)

            # --- x1_att^T[e,q] = sum_k x2n[k,e] * attn[q,k] ---
            for m in range(EC):
                att_ps = psBig.tile([128, L], FP32, tag="big")
                for kc in range(KC):
                    nc.tensor.matmul(
                        att_ps[:], x2n[:, kc, m * 128:(m + 1) * 128], pt_sbs[kc][:],
                        start=(kc == 0), stop=(kc == 3),
                    )
                # --- catT chunks: (x1 - att)^T and (x1 * att)^T ---
                ct_sub = ctpool.tile([128, L], FP32R, tag="ct")
                nc.vector.tensor_tensor(
                    out=ct_sub[:], in0=x1t[:, m, :].bitcast(FP32), in1=att_ps[:],
                    op=ALU.subtract)
                ct_mul = ctpool.tile([128, L], FP32R, tag="ct")
                nc.vector.tensor_tensor(
                    out=ct_mul[:], in0=x1t[:, m, :].bitcast(FP32), in1=att_ps[:],
                    op=ALU.mult)
                if m == 0:
                    ct_tiles = []
                ct_tiles.append((ct_sub, ct_mul))

            # --- fusT[e,l] = relu(fusion_w^T @ catT + fb); pool mean/max ---
            j, b_loc = t % 4, t // 4
            for m in range(EC):
                fus_ps = psBig.tile([128, L], FP32, tag="big")
                for c in range(CC):
                    src = ct_tiles[c % EC][c // EC]
                    nc.tensor.matmul(
                        fus_ps[:], fw_sb[:, c, m * 128:(m + 1) * 128], src[:],
                        start=(c == 0), stop=(c == CC - 1),
                    )
                scr = scpool.tile([128, L], FP32, tag="scr")
                mean_col = (j * 4 + m) * 4 + b_loc
                max_col = (16 + j * 4 + m) * 4 + b_loc
                nc.scalar.activation(
                    scr[:], fus_ps[:], AF.Relu, bias=fb_sb[:, m:m + 1], scale=1.0,
                    accum_out=pooledT[:, mean_col:mean_col + 1],
                )
                nc.vector.reduce_max(
                    pooledT[:, max_col:max_col + 1], scr[:],
                    axis=mybir.AxisListType.X)

            # --- after the last row of group j: its slice of the out matmul ---
            if b_loc == 3:
                nc.vector.tensor_copy(
                    pooledT_r[:, 16 * j:16 * j + 16],
                    pooledT[:, 16 * j:16 * j + 16])
                nc.vector.tensor_copy(
                    pooledT_r[:, 64 + 16 * j:64 + 16 * j + 16],
                    pooledT[:, 64 + 16 * j:64 + 16 * j + 16])
                for gi, gc in enumerate(
                        [j * 4 + ec for ec in range(EC)]
                        + [16 + j * 4 + ec for ec in range(EC)]):
                    ow_sb = owpool.tile([128, 2 * E], FP32R, tag="ow",
                                        name=f"ow_{j}_{gi}")
                    nc.sync.dma_start(ow_sb[:], ow_d[gc * 128:(gc + 1) * 128, :])
                    for h in range(2):
                        nc.tensor.matmul(
                            out_pss[h][:], pooledT_r[:, gc * 4:(gc + 1) * 4],
                            ow_sb[:, h * 512:(h + 1) * 512],
                            start=(j == 0 and gi == 0),
                            stop=(j == 3 and gi == 7),
                        )

        # --- final: out = relu(acc + ob) ---
        for h in range(2):
            o_sb = singles.tile([4, 512], FP32, tag=f"osb{h}")
            nc.vector.tensor_tensor(
                out=o_sb[:], in0=out_pss[h][:], in1=ob_sb[:, h * 512:(h + 1) * 512],
                op=ALU.add)
            nc.vector.tensor_scalar(
                out=o_sb[:], in0=o_sb[:], scalar1=0.0, scalar2=None, op0=ALU.max)
            nc.sync.dma_start(outv_d[:, h * 512:(h + 1) * 512], o_sb[:])

    nc.compile()
    return nc


def _prep_core_inputs(c, x1, x2, x1_len, x2_len, fusion_w, fusion_b, out_w, out_b):
    r0 = 16 * c
    x1_rows = x1[(r0 % B):(r0 % B) + RPC]                      # [16, L, E]
    x2_rows = x2[4 * c:4 * c + 4].reshape(RPC, L, E)           # [16, L, E]
    x1_len_r = np.tile(x1_len, NDOC)[r0:r0 + RPC]              # [16]
    x2_len_r = x2_len.reshape(-1)[r0:r0 + RPC]                 # [16]

    pos = np.arange(L)
    km = (pos[None, :] < x2_len_r[:, None]).astype(np.float32)   # [16, 512]
    qm_rows = (pos[None, :] < x1_len_r[:, None]).astype(np.float32)  # [16, 512]
    # qm[128, 64]: col t*4+qt, partition p -> qm_rows[t, qt*128+p]
    qm = np.ascontiguousarray(
        qm_rows.reshape(RPC, QT, 128).transpose(2, 0, 1).reshape(128, RPC * QT))

    fb = np.ascontiguousarray(fusion_b.reshape(EC, 128).T)     # [128, 4]
    ow = out_w.copy()
    ow[:4 * E] *= np.float32(1.0 / L)                          # fold mean 1/L
    ob = np.broadcast_to(out_b, (4, 2 * E)).copy()

    return {
        "x1t": np.ascontiguousarray(x1_rows.transpose(0, 2, 1)),
        "x2t": np.ascontiguousarray(x2_rows.transpose(0, 2, 1)),
        "x2n": np.ascontiguousarray(x2_rows),
        "km": np.ascontiguousarray(km),
        "qm": qm,
        "fw": np.ascontiguousarray(fusion_w),
        "fb": fb,
        "ow": np.ascontiguousarray(ow),
        "ob": np.ascontiguousarray(ob),
    }


def kernel(x1, x2, x1_len, x2_len, fusion_w, fusion_b, out_w, out_b):
    x1 = np.asarray(x1, dtype=np.float32)
    x2 = np.asarray(x2, dtype=np.float32)
    x1_len = np.asarray(x1_len)
    x2_len = np.asarray(x2_len)
    fusion_w = np.asarray(fusion_w, dtype=np.float32)
    fusion_b = np.asarray(fusion_b, dtype=np.float32)
    out_w = np.asarray(out_w, dtype=np.float32)
    out_b = np.asarray(out_b, dtype=np.float32)

    if "nc" not in _cache:
        _cache["nc"] = build()
    nc = _cache["nc"]

    in_maps = [
        _prep_core_inputs(c, x1, x2, x1_len, x2_len,
                          fusion_w, fusion_b, out_w, out_b)
        for c in range(NCORES)
    ]
    core_ids = list(range(NCORES))
    trace = bool(int(os.environ.get("KERNEL_TRACE", "0")))
    res = run_bass_kernel_spmd(nc, in_maps, core_ids, trace=trace)
    _cache["last_result"] = res

    out = np.concatenate([res.results[c]["outv"] for c in range(NCORES)], axis=0)
    attn = np.concatenate([res.results[c]["attn"] for c in range(NCORES)], axis=0)
    return out, attn
